# revision 6
# baseline (speedup 1.0000x reference)
"""Trainium2 Bass kernel for nn_ActivationFilter:
y = bicubic_down2x( leaky_relu( bicubic_up2x(x) ) ), x: (8, 128, 128, 128) f32 NHWC.

Since jax.image.resize is a separable linear map, per (batch, channel):
    y = D @ leaky_relu(U @ X @ U^T) @ D^T
with U (256x128) the bicubic 2x-upsample matrix and D (128x256) the
antialiased bicubic downsample matrix.

Sharding: batch-per-core (8 batches over 8 NeuronCores), no collectives.

Per-core algorithm (per channel c, all on TensorE, no transposes):
  Ph1: Z1t[w, h2]  = sum_h  x[h, w, c] * Ut[h, h2]       (lhsT = X_c, f32r)
  Ph2: z[h2, w2]   = sum_w  Z1t[w, h2] * Ut[w, w2]       (lhsT = Z1t half, bf16)
       zs = leaky_relu(z)  (fused into PSUM evacuation on ScalarE)
  Ph3: y3[w2, h3]  = sum_h2 zs[h2, w2] * Dt[h2, h3]      (lhsT = zs slice, bf16)
  Ph4: y[w3, h3]   = sum_w2 Dt[w2, w3]^T-form @ y3       (lhsT = Dw, bf16)
Each phase's output partition dim is the next phase's contraction dim,
so no transposes are ever needed.
"""

import sys
import os

if "/opt/trn_rl_repo" not in sys.path:
    sys.path.insert(0, "/opt/trn_rl_repo")

import numpy as np

H = W = C = 128
H2 = W2 = 256
NEG_SLOPE = 0.01


def _keys_cubic(t):
    t = np.abs(t)
    return np.where(
        t <= 1,
        (1.5 * t - 2.5) * t * t + 1,
        np.where(t < 2, ((-0.5 * t + 2.5) * t - 4) * t + 2, 0.0),
    )


def _resize_mat(n_in, n_out, antialias=True):
    """Replicates jax.image.resize(method='bicubic', antialias=True) weights.
    Returns (n_out, n_in) f32 so that y = Wmat @ x along the resized dim."""
    scale = n_out / n_in
    inv_scale = 1.0 / scale
    kernel_scale = max(inv_scale, 1.0) if antialias else 1.0
    sample_f = (np.arange(n_out, dtype=np.float64) + 0.5) * inv_scale - 0.5
    x = (
        np.abs(sample_f[:, None] - np.arange(n_in, dtype=np.float64)[None, :])
        / kernel_scale
    )
    w = _keys_cubic(x)
    total = w.sum(axis=1, keepdims=True)
    w = np.where(np.abs(total) > 1000 * np.finfo(np.float32).eps, w / total, 0)
    w = np.where(((sample_f >= -0.5) & (sample_f <= n_in - 0.5))[:, None], w, 0)
    return w.astype(np.float32)


_BUILD_CACHE = {}


def _build_module():
    """Build + compile the single-core Bass program (same program on all cores)."""
    if "nc" in _BUILD_CACHE:
        return _BUILD_CACHE["nc"]

    import concourse.bacc as bacc
    import concourse.mybir as mybir
    import concourse.tile as tile

    dt = mybir.dt

    nc = bacc.Bacc("TRN2", target_bir_lowering=False, debug=False)

    xin = nc.dram_tensor("xin", (H, W * C), dt.float32r, kind="ExternalInput").ap()
    wh = nc.dram_tensor("wh", (H, H2), dt.float32r, kind="ExternalInput").ap()
    ww = nc.dram_tensor("ww", (W, W2), dt.bfloat16, kind="ExternalInput").ap()
    dh = nc.dram_tensor("dh", (2, 128, 128), dt.bfloat16, kind="ExternalInput").ap()
    dw = nc.dram_tensor("dw", (2, 128, 128), dt.bfloat16, kind="ExternalInput").ap()
    yout = nc.dram_tensor("yout", (W, H * C), dt.float32, kind="ExternalOutput").ap()

    AFT = mybir.ActivationFunctionType

    with tile.TileContext(nc) as tc:
        with (
            tc.tile_pool(name="big", bufs=1) as bigpool,
            tc.tile_pool(name="const", bufs=1) as cpool,
            tc.tile_pool(name="work", bufs=3) as wpool,
            tc.tile_pool(name="ps1", bufs=2, space="PSUM") as ps1,
            tc.tile_pool(name="ps2", bufs=2, space="PSUM") as ps2,
            tc.tile_pool(name="ps3", bufs=2, space="PSUM") as ps3,
            tc.tile_pool(name="ps4", bufs=2, space="PSUM") as ps4,
        ):
            x_sb = bigpool.tile([H, W * C], dt.float32r)
            y_sb = bigpool.tile([W, H * C], dt.float32)
            wh_sb = cpool.tile([H, H2], dt.float32r)
            ww_sb = cpool.tile([W, W2], dt.bfloat16)
            dh_sb = cpool.tile([128, 256], dt.bfloat16)
            dw_sb = cpool.tile([128, 256], dt.bfloat16)

            nc.sync.dma_start(out=wh_sb[:], in_=wh[:])
            nc.sync.dma_start(out=ww_sb[:], in_=ww[:])
            nc.sync.dma_start(out=dh_sb[:, 0:128], in_=dh[0])
            nc.sync.dma_start(out=dh_sb[:, 128:256], in_=dh[1])
            nc.sync.dma_start(out=dw_sb[:, 0:128], in_=dw[0])
            nc.sync.dma_start(out=dw_sb[:, 128:256], in_=dw[1])
            nc.sync.dma_start(out=x_sb[:], in_=xin[:])

            x_r = x_sb[:].rearrange("p (w c) -> p w c", c=C)
            y_r = y_sb[:].rearrange("p (h c) -> p h c", c=C)

            for c in range(C):
                # Ph1: Z1t (w, h2) = X_c^T @ Uh^T   [f32r full-rate, N=256]
                xc = x_r[:, :, c]  # (h, w) strided lhsT, float32r
                p1t = ps1.tile([128, 256], dt.float32)
                nc.tensor.matmul(p1t[:], lhsT=xc, rhs=wh_sb[:], start=True, stop=True)
                z1 = wpool.tile([128, 256], dt.bfloat16, tag="z1")
                nc.vector.tensor_copy(out=z1[:], in_=p1t[:])

                # Ph2: z (h2, w2), two h2-halves side by side in one PSUM bank
                p2t = ps2.tile([128, 512], dt.float32)
                nc.tensor.matmul(
                    p2t[:, 0:256], lhsT=z1[:, 0:128], rhs=ww_sb[:], start=True, stop=True
                )
                nc.tensor.matmul(
                    p2t[:, 256:512], lhsT=z1[:, 128:256], rhs=ww_sb[:], start=True, stop=True
                )
                # leaky_relu fused into evacuation: zs = max(z*0.01, z)
                zs = wpool.tile([128, 512], dt.bfloat16, tag="zs")
                leaky_mode = os.environ.get("AF_LEAKY", "prelu")
                if leaky_mode == "prelu":
                    nc.scalar.activation(zs[:], p2t[:], AFT.Prelu, alpha=NEG_SLOPE)
                elif leaky_mode == "stt":
                    nc.vector.scalar_tensor_tensor(
                        out=zs[:],
                        in0=p2t[:],
                        scalar=NEG_SLOPE,
                        in1=p2t[:],
                        op0=mybir.AluOpType.mult,
                        op1=mybir.AluOpType.max,
                    )
                else:  # "2op"
                    ztmp = wpool.tile([128, 512], dt.bfloat16, tag="ztmp")
                    nc.vector.tensor_scalar_mul(out=ztmp[:], in0=p2t[:], scalar1=NEG_SLOPE)
                    nc.vector.tensor_max(out=zs[:], in0=p2t[:], in1=ztmp[:])

                # Ph3: y3 (w2, h3): for each w2-half a, accumulate over h2-halves b
                p3t = ps3.tile([128, 256], dt.float32)
                for a in range(2):
                    nc.tensor.matmul(
                        p3t[:, a * 128 : a * 128 + 128],
                        lhsT=zs[:, a * 128 : a * 128 + 128],
                        rhs=dh_sb[:, 0:128],
                        start=True,
                        stop=False,
                    )
                    nc.tensor.matmul(
                        p3t[:, a * 128 : a * 128 + 128],
                        lhsT=zs[:, 256 + a * 128 : 256 + a * 128 + 128],
                        rhs=dh_sb[:, 128:256],
                        start=False,
                        stop=True,
                    )
                y3 = wpool.tile([128, 256], dt.bfloat16, tag="y3")
                nc.vector.tensor_copy(out=y3[:], in_=p3t[:])

                # Ph4: y (w3, h3) accumulated over w2-halves
                p4t = ps4.tile([128, 128], dt.float32)
                nc.tensor.matmul(
                    p4t[:], lhsT=dw_sb[:, 0:128], rhs=y3[:, 0:128], start=True, stop=False
                )
                nc.tensor.matmul(
                    p4t[:], lhsT=dw_sb[:, 128:256], rhs=y3[:, 128:256], start=False, stop=True
                )
                nc.vector.tensor_copy(out=y_r[:, :, c], in_=p4t[:])

            nc.sync.dma_start(out=yout[:], in_=y_sb[:])

    nc.compile()
    _BUILD_CACHE["nc"] = nc
    return nc


def _round_tf32(a):
    """Round f32 array to TF32 (10-bit mantissa, RNE) — what FP32R consumes."""
    v = np.ascontiguousarray(a, dtype=np.float32).view(np.uint32)
    lsb = (v >> np.uint32(13)) & np.uint32(1)
    v = v + np.uint32(0x0FFF) + lsb
    v = v & np.uint32(0xFFFFE000)
    return v.view(np.float32)


def _input_maps(x):
    U = _resize_mat(H, H2)   # (256, 128) upsample
    D = _resize_mat(H2, H)   # (128, 256) antialiased downsample
    try:
        from ml_dtypes import bfloat16
    except ImportError:
        import jax.numpy as jnp  # fallback
        bfloat16 = jnp.bfloat16

    wh_np = _round_tf32(np.ascontiguousarray(U.T))         # (h, h2) tf32
    ww_np = np.ascontiguousarray(U.T).astype(bfloat16)     # (w, w2) bf16
    # dh[b, h2local, h3] = D[h3, b*128 + h2local]
    dh_np = np.ascontiguousarray(D.T.reshape(2, 128, 128)).astype(bfloat16)
    dw_np = dh_np.copy()

    in_maps = []
    for i in range(x.shape[0]):
        in_maps.append(
            {
                "xin": _round_tf32(x[i].reshape(H, W * C)),
                "wh": wh_np,
                "ww": ww_np,
                "dh": dh_np,
                "dw": dw_np,
            }
        )
    return in_maps


def _unshard(results):
    outs = []
    for r in results:
        o = np.asarray(r["yout"]).reshape(W, H, C)  # (w3, h3, c)
        outs.append(np.transpose(o, (1, 0, 2)))     # (h3, w3, c)
    return np.stack(outs, axis=0).astype(np.float32)


def run(x, trace=False):
    """Run on 8 NeuronCores. Returns (y, exec_time_ns or None)."""
    from concourse.bass_utils import run_bass_kernel_spmd

    nc = _build_module()
    in_maps = _input_maps(np.asarray(x, dtype=np.float32))
    core_ids = list(range(len(in_maps)))
    res = run_bass_kernel_spmd(nc, in_maps, core_ids, trace=trace)
    return _unshard(res.results), res.exec_time_ns


def kernel(x):
    y, _ = run(x, trace=False)
    return y


def _run_sim(x_batch):
    """CoreSim single-core numerical check (x_batch: (128,128,128) f32)."""
    import concourse.bass_interp as bass_interp

    nc = _build_module()
    sim = bass_interp.CoreSim(nc, trace=False)
    im = _input_maps(x_batch[None])[0]
    for k, v in im.items():
        sim.tensor(k)[:] = v
    sim.simulate()
    o = np.asarray(sim.tensor("yout")).reshape(W, H, C)
    return np.transpose(o, (1, 0, 2))


# revision 10
# speedup vs baseline: 2.1160x; 2.1160x over previous
"""Trainium2 Bass kernel for nn_ActivationFilter:
y = bicubic_down2x( leaky_relu( bicubic_up2x(x) ) ), x: (8, 128, 128, 128) f32 NHWC.

Since jax.image.resize is a separable linear map, per (batch, channel):
    y = D @ leaky_relu(U @ X @ U^T) @ D^T
with U (256x128) the bicubic 2x-upsample matrix and D (128x256) the
antialiased bicubic downsample matrix.

Sharding: batch-per-core (8 batches over 8 NeuronCores), no collectives.

Per-core algorithm (per channel c, all on TensorE, no transposes):
  Ph1: Z1t[w, h2]  = sum_h  x[h, w, c] * Ut[h, h2]       (lhsT = X_c, f32r)
  Ph2: z[h2, w2]   = sum_w  Z1t[w, h2] * Ut[w, w2]       (lhsT = Z1t half, bf16)
       zs = leaky_relu(z)  (fused into PSUM evacuation on ScalarE)
  Ph3: y3[w2, h3]  = sum_h2 zs[h2, w2] * Dt[h2, h3]      (lhsT = zs slice, bf16)
  Ph4: y[w3, h3]   = sum_w2 Dt[w2, w3]^T-form @ y3       (lhsT = Dw, bf16)
Each phase's output partition dim is the next phase's contraction dim,
so no transposes are ever needed.
"""

import sys
import os

if "/opt/trn_rl_repo" not in sys.path:
    sys.path.insert(0, "/opt/trn_rl_repo")

import numpy as np

H = W = C = 128
H2 = W2 = 256
NEG_SLOPE = 0.01


def _keys_cubic(t):
    t = np.abs(t)
    return np.where(
        t <= 1,
        (1.5 * t - 2.5) * t * t + 1,
        np.where(t < 2, ((-0.5 * t + 2.5) * t - 4) * t + 2, 0.0),
    )


def _resize_mat(n_in, n_out, antialias=True):
    """Replicates jax.image.resize(method='bicubic', antialias=True) weights.
    Returns (n_out, n_in) f32 so that y = Wmat @ x along the resized dim."""
    scale = n_out / n_in
    inv_scale = 1.0 / scale
    kernel_scale = max(inv_scale, 1.0) if antialias else 1.0
    sample_f = (np.arange(n_out, dtype=np.float64) + 0.5) * inv_scale - 0.5
    x = (
        np.abs(sample_f[:, None] - np.arange(n_in, dtype=np.float64)[None, :])
        / kernel_scale
    )
    w = _keys_cubic(x)
    total = w.sum(axis=1, keepdims=True)
    w = np.where(np.abs(total) > 1000 * np.finfo(np.float32).eps, w / total, 0)
    w = np.where(((sample_f >= -0.5) & (sample_f <= n_in - 0.5))[:, None], w, 0)
    return w.astype(np.float32)


_BUILD_CACHE = {}


def _build_module():
    """Build + compile the single-core Bass program (same program on all cores)."""
    if "nc" in _BUILD_CACHE:
        return _BUILD_CACHE["nc"]

    import concourse.bacc as bacc
    import concourse.mybir as mybir
    import concourse.tile as tile

    dt = mybir.dt

    nc = bacc.Bacc("TRN2", target_bir_lowering=False, debug=False)

    xin = nc.dram_tensor("xin", (H, W * C), dt.float32r, kind="ExternalInput").ap()
    wh = nc.dram_tensor("wh", (H, H2), dt.float32r, kind="ExternalInput").ap()
    ww = nc.dram_tensor("ww", (W, W2), dt.bfloat16, kind="ExternalInput").ap()
    dh = nc.dram_tensor("dh", (2, 128, 128), dt.bfloat16, kind="ExternalInput").ap()
    dw = nc.dram_tensor("dw", (2, 128, 128), dt.bfloat16, kind="ExternalInput").ap()
    yout = nc.dram_tensor("yout", (W, H * C), dt.float32, kind="ExternalOutput").ap()

    AFT = mybir.ActivationFunctionType

    with tile.TileContext(nc) as tc:
        with (
            tc.tile_pool(name="big", bufs=1) as bigpool,
            tc.tile_pool(name="const", bufs=1) as cpool,
            tc.tile_pool(name="work", bufs=3) as wpool,
            tc.tile_pool(name="ps1", bufs=2, space="PSUM") as ps1,
            tc.tile_pool(name="ps2", bufs=2, space="PSUM") as ps2,
            tc.tile_pool(name="ps3", bufs=1, space="PSUM") as ps3,
            tc.tile_pool(name="ps4", bufs=1, space="PSUM") as ps4,
        ):
            x_sb = bigpool.tile([H, W * C], dt.float32r)
            y_sb = bigpool.tile([W, H * C], dt.float32)
            wh_sb = cpool.tile([H, H2], dt.float32r)
            ww_sb = cpool.tile([W, W2], dt.bfloat16)
            dh_sb = cpool.tile([128, 256], dt.bfloat16)
            dw_sb = cpool.tile([128, 256], dt.bfloat16)

            nc.sync.dma_start(out=wh_sb[:], in_=wh[:])
            nc.sync.dma_start(out=ww_sb[:], in_=ww[:])
            nc.sync.dma_start(out=dh_sb[:, 0:128], in_=dh[0])
            nc.sync.dma_start(out=dh_sb[:, 128:256], in_=dh[1])
            nc.sync.dma_start(out=dw_sb[:, 0:128], in_=dw[0])
            nc.sync.dma_start(out=dw_sb[:, 128:256], in_=dw[1])
            nc.sync.dma_start(out=x_sb[:], in_=xin[:])

            x_r = x_sb[:].rearrange("p (w c) -> p w c", c=C)
            y_r = y_sb[:].rearrange("p (h c) -> p h c", c=C)

            # Process channel PAIRS: two channels share each PSUM tile so every
            # evacuation instruction is 2x larger (per-instruction overheads on
            # DVE/ACT dominate at N=256).
            for cp in range(C // 2):
                c0 = 2 * cp
                # ---- Ph1: Z1t (w, h2) = X_c^T @ Uh^T  [f32r, N=256] ----
                p1t = ps1.tile([128, 512], dt.float32)
                for k in range(2):
                    nc.tensor.matmul(
                        p1t[:, k * 256 : k * 256 + 256],
                        lhsT=x_r[:, :, c0 + k],
                        rhs=wh_sb[:],
                        start=True,
                        stop=True,
                    )
                z1 = wpool.tile([128, 512], dt.bfloat16, tag="z1")
                nc.vector.tensor_copy(out=z1[:], in_=p1t[:])

                # ---- Ph2: z (h2, w2) for both channels, 2 PSUM banks ----
                p2t = ps2.tile([128, 1024], dt.float32)
                for k in range(2):
                    nc.tensor.matmul(
                        p2t[:, k * 512 : k * 512 + 256],
                        lhsT=z1[:, k * 256 : k * 256 + 128],
                        rhs=ww_sb[:],
                        start=True,
                        stop=True,
                    )
                    nc.tensor.matmul(
                        p2t[:, k * 512 + 256 : k * 512 + 512],
                        lhsT=z1[:, k * 256 + 128 : k * 256 + 256],
                        rhs=ww_sb[:],
                        start=True,
                        stop=True,
                    )
                # leaky_relu fused into the PSUM evacuation (ScalarE Prelu)
                zs = wpool.tile([128, 1024], dt.bfloat16, tag="zs")
                if os.environ.get("AF_SIM_RELU", "0") == "1":
                    # CoreSim has no Prelu; plain Relu validates the plumbing
                    nc.scalar.activation(zs[:], p2t[:], AFT.Relu)
                else:
                    nc.scalar.activation(zs[:], p2t[:], AFT.Prelu, alpha=NEG_SLOPE)

                # ---- Ph3: y3 (w2, h3) per channel (zs k-offset: k*512) ----
                p3t = ps3.tile([128, 512], dt.float32)
                for k in range(2):
                    for a in range(2):
                        o = k * 256 + a * 128
                        nc.tensor.matmul(
                            p3t[:, o : o + 128],
                            lhsT=zs[:, k * 512 + a * 128 : k * 512 + a * 128 + 128],
                            rhs=dh_sb[:, 0:128],
                            start=True,
                            stop=False,
                        )
                        nc.tensor.matmul(
                            p3t[:, o : o + 128],
                            lhsT=zs[:, k * 512 + 256 + a * 128 : k * 512 + 256 + a * 128 + 128],
                            rhs=dh_sb[:, 128:256],
                            start=False,
                            stop=True,
                        )
                y3 = wpool.tile([128, 512], dt.bfloat16, tag="y3")
                if cp % 2 == 0:  # split e3 across ACT/DVE to balance engines
                    nc.scalar.activation(y3[:], p3t[:], AFT.Copy)
                else:
                    nc.vector.tensor_copy(out=y3[:], in_=p3t[:])

                # ---- Ph4: y (w3, h3) per channel ----
                p4t = ps4.tile([128, 256], dt.float32)
                for k in range(2):
                    nc.tensor.matmul(
                        p4t[:, k * 128 : k * 128 + 128],
                        lhsT=dw_sb[:, 0:128],
                        rhs=y3[:, k * 256 : k * 256 + 128],
                        start=True,
                        stop=False,
                    )
                    nc.tensor.matmul(
                        p4t[:, k * 128 : k * 128 + 128],
                        lhsT=dw_sb[:, 128:256],
                        rhs=y3[:, k * 256 + 128 : k * 256 + 256],
                        start=False,
                        stop=True,
                    )
                nc.vector.tensor_copy(out=y_r[:, :, c0 : c0 + 2], in_=p4t[:].rearrange("p (k n) -> p n k", k=2))

            nc.sync.dma_start(out=yout[:], in_=y_sb[:])

    nc.compile()
    _BUILD_CACHE["nc"] = nc
    return nc


def _round_tf32(a):
    """Round f32 array to TF32 (10-bit mantissa, RNE) — what FP32R consumes."""
    v = np.ascontiguousarray(a, dtype=np.float32).view(np.uint32)
    lsb = (v >> np.uint32(13)) & np.uint32(1)
    v = v + np.uint32(0x0FFF) + lsb
    v = v & np.uint32(0xFFFFE000)
    return v.view(np.float32)


def _input_maps(x):
    U = _resize_mat(H, H2)   # (256, 128) upsample
    D = _resize_mat(H2, H)   # (128, 256) antialiased downsample
    try:
        from ml_dtypes import bfloat16
    except ImportError:
        import jax.numpy as jnp  # fallback
        bfloat16 = jnp.bfloat16

    wh_np = _round_tf32(np.ascontiguousarray(U.T))         # (h, h2) tf32
    ww_np = np.ascontiguousarray(U.T).astype(bfloat16)     # (w, w2) bf16
    # dh[b, h2local, h3] = D[h3, b*128 + h2local]
    dh_np = np.ascontiguousarray(D.T.reshape(2, 128, 128)).astype(bfloat16)
    dw_np = dh_np.copy()

    in_maps = []
    for i in range(x.shape[0]):
        in_maps.append(
            {
                "xin": _round_tf32(x[i].reshape(H, W * C)),
                "wh": wh_np,
                "ww": ww_np,
                "dh": dh_np,
                "dw": dw_np,
            }
        )
    return in_maps


def _unshard(results):
    outs = []
    for r in results:
        o = np.asarray(r["yout"]).reshape(W, H, C)  # (w3, h3, c)
        outs.append(np.transpose(o, (1, 0, 2)))     # (h3, w3, c)
    return np.stack(outs, axis=0).astype(np.float32)


def run(x, trace=False):
    """Run on 8 NeuronCores. Returns (y, exec_time_ns or None)."""
    from concourse.bass_utils import run_bass_kernel_spmd

    nc = _build_module()
    in_maps = _input_maps(np.asarray(x, dtype=np.float32))
    core_ids = list(range(len(in_maps)))
    res = run_bass_kernel_spmd(nc, in_maps, core_ids, trace=trace)
    return _unshard(res.results), res.exec_time_ns


def kernel(x):
    y, _ = run(x, trace=False)
    return y


def _run_sim(x_batch):
    """CoreSim single-core numerical check (x_batch: (128,128,128) f32)."""
    import concourse.bass_interp as bass_interp

    nc = _build_module()
    sim = bass_interp.CoreSim(nc, trace=False)
    im = _input_maps(x_batch[None])[0]
    for k, v in im.items():
        sim.tensor(k)[:] = v
    sim.simulate()
    o = np.asarray(sim.tensor("yout")).reshape(W, H, C)
    return np.transpose(o, (1, 0, 2))
